# revision 20
# baseline (speedup 1.0000x reference)
"""BinsEdgeAccuracyLoss Trainium2 Bass kernel.

Math background
---------------
The reference loops over 8 uniform bins on [-1, 1] and counts elements where
input x lies in bin j (lower-open, upper-closed) AND target t equals
np.linspace(-1, 1, 8)[j] exactly (float32 equality), plus an edge term for
x == -1 with t == -1.  That whole computation reduces to one per-element
predicate:

    match  <=>  t == npdvals[bucket(x)]

where bucket(-1) folds into bucket 0 (covering the edge term), and npdvals
are the float32 values of np.linspace(-1, 1, 8) (computed in float64, cast
to f32).  Targets are built from jnp.linspace, which does NOT bitwise-match
np.linspace at every index, so the comparison constants must be the numpy
ones; the kernel reconstructs npdvals[bucket(x)] *bit-exactly* per element
and compares against t at full f32 resolution, making the kernel correct for
any target bit patterns (platform-independent).

Per-element pipeline (validated exhaustively on host for every representable
input value - the inputs are multiples of 2^-22 in [-1, 1)):

  ACT:  zx  = Copy(4*x - (0.5 + 2^-21))            # exact in f32
  ACT:  wxr = Copy(zx + M3), M3 = 1.5*2^23         # RNE onto integer grid
  DVE:  k   = max(wxr, M3-4) - M3                  # bucket j-4; x==-1 -> -4
  DVE:  itm = (k + 0.5) * C1A                      # C1A ~ (2/7)/64
  DVE:  out = (itm * C1B) == t ; accum = sum(out)  # C1B ~ 64; the double-
                                                   # rounded product equals
                                                   # npdvals[j] bit-exactly
The scalar_tensor_tensor instruction fuses the final multiply, the equality
compare against t, and the free-dim sum reduction in one DVE pass.

Sharding: 4096 rows split 512/core across 8 cores (data parallel).  Each
core returns [128, N_TILES] f32 partial counts; the host sums them and forms
the loss with the same f32 arithmetic as the reference.

Note: built on bacc.Bacc (not raw bass.Bass) - TRN2 instructions can carry
at most one semaphore wait and Bacc's generate_event_semaphores pass splits
multi-wait instructions automatically.
"""

import numpy as np

N0, N1 = 4096, 16384
N_CORES = 8
ROWS_PER_CORE = N0 // N_CORES          # 512
ROW_GROUPS = ROWS_PER_CORE // 128      # 4
COL_CHUNK = 4096
COL_CHUNKS = N1 // COL_CHUNK           # 4
N_TILES = ROW_GROUPS * COL_CHUNKS      # 16

# Bit-exact constants (see module docstring); all round-trip exactly to f32.
BIAS1 = -0.5000004768371582            # -(0.5 + 2^-21)
M3 = 12582912.0                        # 1.5 * 2^23
M0 = 12582908.0                        # M3 - 4
C1A = 0.004464286845177412             # bits 0x3B925325
C1B = 63.99998474121094                # bits 0x427FFFFC

_cached = {}


def _build_program():
    import concourse.bacc as bacc
    import concourse.mybir as mybir
    from concourse.tile import TileContext

    f32 = mybir.dt.float32
    nc = bacc.Bacc()
    x = nc.dram_tensor("x", [ROWS_PER_CORE, N1], f32, kind="ExternalInput")
    t = nc.dram_tensor("t", [ROWS_PER_CORE, N1], f32, kind="ExternalInput")
    out = nc.dram_tensor("partials", [128, N_TILES], f32, kind="ExternalOutput")

    with TileContext(nc) as tc:
        with (
            tc.tile_pool(name="xp", bufs=3) as xpool,
            tc.tile_pool(name="tp", bufs=3) as tpool,
            tc.tile_pool(name="accp", bufs=1) as accpool,
        ):
            acc = accpool.tile([128, N_TILES], f32)
            for i in range(N_TILES):
                g, cc = divmod(i, COL_CHUNKS)
                xt = xpool.tile([128, COL_CHUNK], f32)
                tt = tpool.tile([128, COL_CHUNK], f32)
                rows = slice(g * 128, (g + 1) * 128)
                cols = slice(cc * COL_CHUNK, (cc + 1) * COL_CHUNK)
                nc.sync.dma_start(out=xt[:], in_=x[rows, cols])
                nc.sync.dma_start(out=tt[:], in_=t[rows, cols])
                nc.scalar.activation(
                    xt[:], xt[:], mybir.ActivationFunctionType.Copy,
                    bias=BIAS1, scale=4.0,
                )
                nc.scalar.activation(
                    xt[:], xt[:], mybir.ActivationFunctionType.Copy,
                    bias=M3, scale=1.0,
                )
                nc.vector.tensor_scalar(
                    xt[:], xt[:], M0, M3,
                    op0=mybir.AluOpType.max, op1=mybir.AluOpType.subtract,
                )
                nc.vector.tensor_scalar(
                    xt[:], xt[:], 0.5, C1A,
                    op0=mybir.AluOpType.add, op1=mybir.AluOpType.mult,
                )
                nc.vector.scalar_tensor_tensor(
                    xt[:], xt[:], C1B, tt[:],
                    op0=mybir.AluOpType.mult, op1=mybir.AluOpType.is_equal,
                    accum_out=acc[:, i : i + 1],
                )
            nc.sync.dma_start(out=out[:], in_=acc[:])
    nc.finalize()  # runs Bacc.compile(): reg alloc + multi-wait splitting
    return nc


def kernel(input, target, bins):
    from concourse.bass_utils import run_bass_kernel_spmd

    if "nc" not in _cached:
        _cached["nc"] = _build_program()
    nc = _cached["nc"]

    x = np.ascontiguousarray(np.asarray(input, dtype=np.float32))
    t = np.ascontiguousarray(np.asarray(target, dtype=np.float32))

    in_maps = []
    for c in range(N_CORES):
        rows = slice(c * ROWS_PER_CORE, (c + 1) * ROWS_PER_CORE)
        in_maps.append({"x": x[rows], "t": t[rows]})

    res = run_bass_kernel_spmd(nc, in_maps, list(range(N_CORES)))
    count = 0
    for c in range(N_CORES):
        count += int(np.sum(res.results[c]["partials"].astype(np.float64)))

    numel = N0 * N1
    edge_acc = np.float32(np.float32(count) / np.float32(numel))
    loss = np.float32(np.float32(1.0) - edge_acc)
    return np.array(loss, dtype=np.float32)


# revision 21
# speedup vs baseline: 1.0424x; 1.0424x over previous
"""BinsEdgeAccuracyLoss Trainium2 Bass kernel.

Math background
---------------
The reference loops over 8 uniform bins on [-1, 1] and counts elements where
input x lies in bin j (lower-open, upper-closed) AND target t equals
np.linspace(-1, 1, 8)[j] exactly (float32 equality), plus an edge term for
x == -1 with t == -1.  That whole computation reduces to one per-element
predicate:

    match  <=>  t == npdvals[bucket(x)]

where bucket(-1) folds into bucket 0 (covering the edge term), and npdvals
are the float32 values of np.linspace(-1, 1, 8) (computed in float64, cast
to f32).  Targets are built from jnp.linspace, which does NOT bitwise-match
np.linspace at every index, so the comparison constants must be the numpy
ones; the kernel reconstructs npdvals[bucket(x)] *bit-exactly* per element
and compares against t at full f32 resolution, making the kernel correct for
any target bit patterns (platform-independent).

Per-element pipeline (validated exhaustively on host for every representable
input value - the inputs are multiples of 2^-22 in [-1, 1)):

  ACT:  zx  = Copy(4*x - (0.5 + 2^-21))            # exact in f32
  ACT:  wxr = Copy(zx + M3), M3 = 1.5*2^23         # RNE onto integer grid
  DVE:  k   = max(wxr, M3-4) - M3                  # bucket j-4; x==-1 -> -4
  DVE:  itm = (k + 0.5) * C1A                      # C1A ~ (2/7)/64
  DVE:  out = (itm * C1B) == t ; accum = sum(out)  # C1B ~ 64; the double-
                                                   # rounded product equals
                                                   # npdvals[j] bit-exactly
The scalar_tensor_tensor instruction fuses the final multiply, the equality
compare against t, and the free-dim sum reduction in one DVE pass.

Sharding: 4096 rows split 512/core across 8 cores (data parallel).  Each
core returns [128, N_TILES] f32 partial counts; the host sums them and forms
the loss with the same f32 arithmetic as the reference.

Note: built on bacc.Bacc (not raw bass.Bass) - TRN2 instructions can carry
at most one semaphore wait and Bacc's generate_event_semaphores pass splits
multi-wait instructions automatically.
"""

import numpy as np

N0, N1 = 4096, 16384
N_CORES = 8
ROWS_PER_CORE = N0 // N_CORES          # 512
ROW_GROUPS = ROWS_PER_CORE // 128      # 4
COL_CHUNK = 512
COL_CHUNKS = N1 // COL_CHUNK           # 32
N_TILES = ROW_GROUPS * COL_CHUNKS      # 128

# Bit-exact constants (see module docstring); all round-trip exactly to f32.
BIAS1 = -0.5000004768371582            # -(0.5 + 2^-21)
M3 = 12582912.0                        # 1.5 * 2^23
M0 = 12582908.0                        # M3 - 4
C1A = 0.004464286845177412             # bits 0x3B925325
C1B = 63.99998474121094                # bits 0x427FFFFC

_cached = {}


def _build_program():
    import concourse.bacc as bacc
    import concourse.mybir as mybir
    from concourse.tile import TileContext

    f32 = mybir.dt.float32
    nc = bacc.Bacc()
    x = nc.dram_tensor("x", [ROWS_PER_CORE, N1], f32, kind="ExternalInput")
    t = nc.dram_tensor("t", [ROWS_PER_CORE, N1], f32, kind="ExternalInput")
    out = nc.dram_tensor("partials", [128, N_TILES], f32, kind="ExternalOutput")

    with TileContext(nc) as tc:
        with (
            tc.tile_pool(name="xp", bufs=8) as xpool,
            tc.tile_pool(name="tp", bufs=8) as tpool,
            tc.tile_pool(name="accp", bufs=1) as accpool,
        ):
            acc = accpool.tile([128, N_TILES], f32)
            for i in range(N_TILES):
                g, cc = divmod(i, COL_CHUNKS)
                xt = xpool.tile([128, COL_CHUNK], f32)
                tt = tpool.tile([128, COL_CHUNK], f32)
                rows = slice(g * 128, (g + 1) * 128)
                cols = slice(cc * COL_CHUNK, (cc + 1) * COL_CHUNK)
                nc.sync.dma_start(out=xt[:], in_=x[rows, cols])
                nc.sync.dma_start(out=tt[:], in_=t[rows, cols])
                nc.scalar.activation(
                    xt[:], xt[:], mybir.ActivationFunctionType.Copy,
                    bias=BIAS1, scale=4.0,
                )
                nc.scalar.activation(
                    xt[:], xt[:], mybir.ActivationFunctionType.Copy,
                    bias=M3, scale=1.0,
                )
                nc.vector.tensor_scalar(
                    xt[:], xt[:], M0, M3,
                    op0=mybir.AluOpType.max, op1=mybir.AluOpType.subtract,
                )
                nc.vector.tensor_scalar(
                    xt[:], xt[:], 0.5, C1A,
                    op0=mybir.AluOpType.add, op1=mybir.AluOpType.mult,
                )
                nc.vector.scalar_tensor_tensor(
                    xt[:], xt[:], C1B, tt[:],
                    op0=mybir.AluOpType.mult, op1=mybir.AluOpType.is_equal,
                    accum_out=acc[:, i : i + 1],
                )
            nc.sync.dma_start(out=out[:], in_=acc[:])
    nc.finalize()  # runs Bacc.compile(): reg alloc + multi-wait splitting
    return nc


def kernel(input, target, bins):
    from concourse.bass_utils import run_bass_kernel_spmd

    if "nc" not in _cached:
        _cached["nc"] = _build_program()
    nc = _cached["nc"]

    x = np.ascontiguousarray(np.asarray(input, dtype=np.float32))
    t = np.ascontiguousarray(np.asarray(target, dtype=np.float32))

    in_maps = []
    for c in range(N_CORES):
        rows = slice(c * ROWS_PER_CORE, (c + 1) * ROWS_PER_CORE)
        in_maps.append({"x": x[rows], "t": t[rows]})

    res = run_bass_kernel_spmd(nc, in_maps, list(range(N_CORES)))
    count = 0
    for c in range(N_CORES):
        count += int(np.sum(res.results[c]["partials"].astype(np.float64)))

    numel = N0 * N1
    edge_acc = np.float32(np.float32(count) / np.float32(numel))
    loss = np.float32(np.float32(1.0) - edge_acc)
    return np.array(loss, dtype=np.float32)
